# revision 18
# baseline (speedup 1.0000x reference)
"""Single-head causal attention (B=8, T=2048, D=512, H=64) on 8 TRN2 cores.

Data-parallel: one batch element per NeuronCore. Each core computes
attention in the S^T layout (keys on partitions, queries on the free axis):

  qT/kT/vT [64, T] = W.T @ x.T        (fp16 matmuls, c-tile chunks)
  v        [T, 64] via PE transpose of vT, with a ones column appended
  S^T[j,i] = kT_jblock.T @ qT          (strips of causal width)
  P^T      = exp(S^T / 8)              (ScalarE; no max-subtraction:
                                        scores bounded well below fp16 max)
  out^T[h,i], l[i] = [v|1]_jb.T @ P^T  (accumulated over j-blocks in PSUM;
                                        row 64 is the softmax denominator)

The kernel returns the unnormalized [65, T] strip per core; the host
divides by the denominator row and transposes back to [T, 64].

All inputs ship in ONE contiguous DRAM blob [128, 9168] f16
([consts | x-plane tc0..tc3], tc-major x so rows are 2-4KB contiguous),
split into six sequential DMAs ordered so the first projection's
dependencies land first.
"""

import sys

sys.path.insert(0, "/opt/trn_rl_repo")

import numpy as np

import concourse.bass as bass
import concourse.mybir as mybir
import concourse.tile as tile

B, T, D, H = 8, 2048, 512, 64
N_CORES = 8
HALF = T // 2  # i-axis pass width
CONSTS_W = 976  # [wqk c0..c3 | wv c0..c3 | mask | ones | ident]
BLOB_W = CONSTS_W + 4 * 2048  # + four tc-major x planes [4c, 512]

f32 = mybir.dt.float32
f16 = mybir.dt.float16

_cache = {}


def _legalize_waits(nc, max_waits=1):
    """Walrus codegen accepts at most one sync wait per instruction; hoist
    extras onto same-engine NOPs placed immediately before (engine queues
    are FIFO so blocking semantics are unchanged)."""
    counter = 0
    for bb in nc.main_func.blocks:
        if not any(
            ins.sync_info is not None and len(ins.sync_info.on_wait) > max_waits
            for ins in bb.instructions
        ):
            continue
        new_list = []
        for ins in bb.instructions:
            si = ins.sync_info
            if si is not None and len(si.on_wait) > max_waits:
                waits = list(si.on_wait)
                hoist, keep = waits[:-max_waits], waits[-max_waits:]
                for w in hoist:
                    counter += 1
                    new_list.append(
                        mybir.InstNoOp(
                            name=f"I-waitfix-{counter}",
                            engine=ins.engine,
                            sync_info=mybir.SyncInfo(on_wait=[w], on_update=[]),
                            bass_nofuse=True,
                        )
                    )
                ins.sync_info = mybir.SyncInfo(
                    on_wait=keep, on_update=list(si.on_update)
                )
            new_list.append(ins)
        bb.instructions = new_list
    return counter


def _chunks(lo, hi, step, align):
    """Split [lo, hi) at multiples of `step` relative to `align`."""
    out = []
    cur = lo
    while cur < hi:
        nxt = min(hi, align + ((cur - align) // step + 1) * step)
        out.append((cur, nxt))
        cur = nxt
    return out


def _build():
    nc = bass.Bass()

    blob_d = nc.declare_dram_parameter("blob", [128, BLOB_W], f16, isOutput=False)
    out_d = nc.declare_dram_parameter("out", [H + 1, T], f32, isOutput=True)

    NC_TILES = D // 128  # 4 c-tiles

    with tile.TileContext(nc) as tc_ctx:
        with (
            tc_ctx.tile_pool(name="const", bufs=1) as cpool,
            tc_ctx.tile_pool(name="qkv", bufs=1) as qkvpool,
            tc_ctx.tile_pool(name="p", bufs=2) as ppool,
            tc_ctx.tile_pool(name="o", bufs=2) as opool,
            tc_ctx.tile_pool(name="ps_proj", bufs=2, space="PSUM") as ps_proj,
            tc_ctx.tile_pool(name="ps_s", bufs=2, space="PSUM") as ps_s,
            tc_ctx.tile_pool(name="ps_pv", bufs=1, space="PSUM") as ps_pv,
        ):
            blob = cpool.tile([128, BLOB_W], f16)

            def xcol(tc512, c, lo=0, hi=512):
                base = CONSTS_W + 4 * tc512 + 512 * c
                return blob[:, base + lo : base + hi]

            # consts then per-tc x planes, split across two queues with
            # just-in-time arrival: a PE onset before ~12us makes the HAM
            # controller grant only a short fragmented boost window, so
            # don't deliver data faster than the projections consume it
            cuts = [0, CONSTS_W, CONSTS_W + 2048, CONSTS_W + 4096,
                    CONSTS_W + 6144, BLOB_W]
            for qi, (lo, hi) in enumerate(zip(cuts[:-1], cuts[1:])):
                eng = nc.sync if qi < 3 else nc.gpsimd
                eng.dma_start(blob[:, lo:hi], blob_d[:, lo:hi])

            wqk = [blob[:, 128 * c : 128 * (c + 1)] for c in range(NC_TILES)]
            wv = [blob[:, 512 + 64 * c : 512 + 64 * (c + 1)] for c in range(NC_TILES)]
            mask16 = blob[:, 768:896]
            ones = blob[:, 896:912]
            ident16 = blob[0:H, 912:976]

            # Touch Exp so the ACT table set loads now (1.3us) instead of on
            # the first real strip. No PE warm-up: the clock-boost budget is
            # limited and idle PE time before the real work banks credit.
            warm_bf = cpool.tile([128, 4], mybir.dt.bfloat16)
            nc.vector.memset(warm_bf[:], 1.0)
            exp_warm = cpool.tile([1, 2], f32)
            nc.scalar.activation(
                exp_warm[:], warm_bf[0:1, 0:2], mybir.ActivationFunctionType.Exp
            )

            qT = qkvpool.tile([H, T], f16)
            kT = qkvpool.tile([H, T], f16)
            vT = qkvpool.tile([H, T], f16)
            v1 = qkvpool.tile([128, T // 128, H + 1], f16)
            nc.vector.tensor_copy(v1[:, :, H : H + 1], ones)

            def proj_qk_unit(tc512):
                qk_ps = ps_proj.tile([128, 512], f32, tag="work", name="qk_ps")
                for c in range(NC_TILES):
                    nc.tensor.matmul(
                        qk_ps[:],
                        wqk[c],
                        xcol(tc512, c),
                        start=(c == 0),
                        stop=(c == NC_TILES - 1),
                    )
                with tc_ctx.high_priority(offset=50000):
                    nc.vector.tensor_copy(
                        kT[:, tc512 : tc512 + 512], qk_ps[H : 2 * H, :]
                    )
                    nc.vector.tensor_copy(
                        qT[:, tc512 : tc512 + 512], qk_ps[0:H, :]
                    )

            def proj_v_unit(tc512):
                v_ps = ps_proj.tile([128, 512], f32, tag="work", name="v_ps")
                for c in range(NC_TILES):
                    nc.tensor.matmul(
                        v_ps[0:H, :],
                        wv[c],
                        xcol(tc512, c),
                        start=(c == 0),
                        stop=(c == NC_TILES - 1),
                    )
                nc.vector.tensor_copy(vT[:, tc512 : tc512 + 512], v_ps[0:H, :])

            def vtrans_unit(jj_pair):
                vt_ps = ps_proj.tile([128, 2, H], f16, tag="work", name="vt_ps")
                for jl, jj in enumerate(jj_pair):
                    nc.tensor.transpose(
                        vt_ps[:, jl, :],
                        vT[:, 128 * jj : 128 * (jj + 1)],
                        ident16,
                    )
                    nc.vector.tensor_copy(v1[:, jj, 0:H], vt_ps[:, jl, :])

            def s_matmul(s_ps, jb, i_start, off, lo, hi):
                # strip cols [i_start+lo, i_start+hi) into s_ps[off+lo:off+hi]
                for ls, le in _chunks(off + lo, off + hi, 512, 0):
                    nc.tensor.matmul(
                        s_ps[:, ls:le],
                        kT[:, 128 * jb : 128 * (jb + 1)],
                        qT[:, i_start - off + ls : i_start - off + le],
                        start=True,
                        stop=True,
                    )

            # A strip group: one ps_s tile holding the strips of one or two
            # j-blocks side by side, consumed by ONE exp instruction.
            def strip_group(t0, jbs, lim=None):
                s_ps = ps_s.tile([128, HALF], f32, tag="s", name="s_ps")
                off = 0
                offs = {}
                for jb in jbs:
                    i_start = max(t0, 128 * jb)
                    W = t0 + HALF - i_start
                    offs[jb] = (off, i_start, W)
                    s_matmul(s_ps, jb, i_start, off, 0, min(W, lim) if lim else W)
                    off += W
                return s_ps, offs

            def exp_group(s_ps, offs, t0, hi_lim=None, p_sb=None, lo_lim=0):
                total = max(o + w for (o, _, w) in offs.values())
                if p_sb is None:
                    p_sb = ppool.tile([128, HALF], f16, tag="p", name="p_sb", bufs=6)
                nc.scalar.activation(
                    p_sb[:, lo_lim : hi_lim if hi_lim else total],
                    s_ps[:, lo_lim : hi_lim if hi_lim else total],
                    mybir.ActivationFunctionType.Exp,
                    scale=1.0 / 8.0,
                )
                if lo_lim == 0:
                    for jb, (off, i_start, W) in offs.items():
                        if 128 * jb >= t0:
                            # gpsimd is idle; keep the diagonal mask off DVE
                            nc.gpsimd.tensor_mul(
                                p_sb[:, off : off + 128],
                                p_sb[:, off : off + 128],
                                mask16,
                            )
                return p_sb

            def exp_store(t0, jb):
                # pass-1 strip precomputed during pass 0 into a held P slot
                s_ps, offs = strip_group(t0, (jb,))
                W = offs[jb][2]
                p_sb = ppool.tile([128, HALF], f16, tag="ppre", name="p_pre", bufs=6)
                nc.scalar.activation(
                    p_sb[:, 0:W],
                    s_ps[:, 0:W],
                    mybir.ActivationFunctionType.Exp,
                    scale=1.0 / 8.0,
                )
                return p_sb

            def attn_pv(t0, n_jb, pv_ps, jb, p_sb, off=0, i_start=None):
                if i_start is None:
                    i_start = max(t0, 128 * jb)
                for gs, ge in _chunks(i_start, t0 + HALF, 512, 0):
                    ic_last_jb = min(n_jb - 1, (ge - 1) // 128)
                    nc.tensor.matmul(
                        pv_ps[:, gs - t0 : ge - t0],
                        v1[:, jb, :],
                        p_sb[:, off + gs - i_start : off + ge - i_start],
                        start=(jb == 0),
                        stop=(jb == ic_last_jb),
                    )

            def out_piece(pv_ps, t0, lo, hi):
                out_sb = opool.tile([H + 1, 512], f32, tag="o", name="out_sb")
                nc.vector.tensor_copy(out_sb[:, 0 : hi - lo], pv_ps[:, lo:hi])
                nc.sync.dma_start(
                    out_d[:, t0 + lo : t0 + hi], out_sb[:, 0 : hi - lo]
                )

            # --- startup: first exp as early as possible. S(0,0) cols
            # [0:512) only need proj(tc0); extend to [512:1024) after
            # proj(tc1). ---
            proj_qk_unit(0)
            s00, offs00 = strip_group(0, (0,), lim=512)
            p00 = exp_group(s00, offs00, 0, hi_lim=512)
            proj_v_unit(0)
            proj_qk_unit(512)
            s_matmul(s00, 0, 0, 0, 512, 1024)
            exp_group(s00, offs00, 0, p_sb=p00, lo_lim=512)
            vtrans_unit((0, 1))

            # --- attention pass 0 (i in [0,1024)); proj/vtrans woven in;
            # pass-1 strips precomputed from unit 4 on to keep ACT fed ---
            units0 = [(1,), (2,), (3,), (4,), (5,), (6, 7)]
            weave = {
                0: [lambda: vtrans_unit((2, 3)), lambda: proj_qk_unit(1024)],
                1: [lambda: proj_v_unit(512)],
                2: [lambda: vtrans_unit((4, 5)), lambda: proj_qk_unit(1536)],
                3: [lambda: vtrans_unit((6, 7))],
                5: [lambda: proj_v_unit(1024)],
            }
            pv_ps0 = ps_pv.tile([H + 1, HALF], f32, tag="pv", name="pv_ps")
            pend = [(0, p00, 0, 0)]  # (jb, p_sb, off, i_start)
            pre_p = []
            npre = 6
            for ui, jbs in enumerate(units0):
                s_ps, offs = strip_group(0, jbs)
                for jb, p_sb, off, i_start in pend:
                    attn_pv(0, 8, pv_ps0, jb, p_sb, off, i_start)
                if ui == 3:
                    # PV(3) just emitted: cols [0:512) of pass 0 complete
                    out_piece(pv_ps0, 0, 0, 512)
                p_sb = exp_group(s_ps, offs, 0)
                pend = [(jb, p_sb, offs[jb][0], offs[jb][1]) for jb in jbs]
                for u in weave.get(ui, ()):
                    u()
                if ui >= 3:
                    pre_p.append(exp_store(HALF, len(pre_p)))
            for jb, p_sb, off, i_start in pend:
                attn_pv(0, 8, pv_ps0, jb, p_sb, off, i_start)
            proj_v_unit(1536)
            vtrans_unit((8, 9))
            while len(pre_p) < npre:
                pre_p.append(exp_store(HALF, len(pre_p)))
            out_piece(pv_ps0, 0, 512, 1024)

            # --- attention pass 1 (i in [1024,2048)) ---
            units1 = [(6,), (7,), (8,), (9,), (10,), (11,), (12, 13), (14, 15)]
            weave1 = {
                0: [lambda: vtrans_unit((10, 11))],
                1: [lambda: vtrans_unit((12, 13))],
                2: [lambda: vtrans_unit((14, 15))],
            }
            pv_ps1 = ps_pv.tile([H + 1, HALF], f32, tag="pv", name="pv_ps")
            for jb in range(npre):
                attn_pv(HALF, 16, pv_ps1, jb, pre_p[jb])
            pend = []
            for ui, jbs in enumerate(units1):
                s_ps, offs = strip_group(HALF, jbs)
                for jb, p_sb, off, i_start in pend:
                    attn_pv(HALF, 16, pv_ps1, jb, p_sb, off, i_start)
                if ui == 6:
                    # PV(11) just emitted: cols [1024:1536) complete
                    out_piece(pv_ps1, HALF, 0, 512)
                p_sb = exp_group(s_ps, offs, HALF)
                pend = [(jb, p_sb, offs[jb][0], offs[jb][1]) for jb in jbs]
                for u in weave1.get(ui, ()):
                    u()
            for jb, p_sb, off, i_start in pend:
                attn_pv(HALF, 16, pv_ps1, jb, p_sb, off, i_start)
            out_piece(pv_ps1, HALF, 512, 1024)

    _legalize_waits(nc)
    return nc


def build_in_maps(x, Wq, Wk, Wv):
    x = np.ascontiguousarray(np.asarray(x), dtype=np.float32)
    wqk_np = np.ascontiguousarray(
        np.concatenate([np.asarray(Wq), np.asarray(Wk)], axis=1), dtype=np.float32
    )
    wv_np = np.ascontiguousarray(np.asarray(Wv), dtype=np.float32)

    def ctile_pack(a, w):  # [512, w] -> [128, 4*w] with c-tiles side by side
        return a.reshape(4, 128, w).transpose(1, 0, 2).reshape(128, 4 * w)

    mask_np = np.triu(np.ones((128, 128), dtype=np.float16))
    ident_np = np.zeros((128, H), dtype=np.float16)
    ident_np[:H] = np.eye(H, dtype=np.float16)
    ones_np = np.ones((128, T // 128), dtype=np.float16)
    consts_np = np.concatenate(
        [
            ctile_pack(wqk_np.astype(np.float16), 128),
            ctile_pack(wv_np.astype(np.float16), 64),
            mask_np,
            ones_np,
            ident_np,
        ],
        axis=1,
    )

    maps = []
    for b in range(N_CORES):
        xt = x[b].astype(np.float16)  # [T, D]
        # plane tc: [128p, 4c, 512t'] with xt[512tc+t', 128c+p]
        planes = xt.reshape(4, 512, 4, 128).transpose(0, 3, 2, 1).reshape(4, 128, 2048)
        blob = np.concatenate([consts_np] + [planes[tc] for tc in range(4)], axis=1)
        maps.append({"blob": np.ascontiguousarray(blob)})
    return maps


def kernel(x, Wq, Wk, Wv):
    from concourse.bass_utils import run_bass_kernel_spmd

    if "nc" not in _cache:
        _cache["nc"] = _build()
    nc = _cache["nc"]

    in_maps = build_in_maps(x, Wq, Wk, Wv)
    res = run_bass_kernel_spmd(nc, in_maps, list(range(N_CORES))).results

    out = np.empty((B, T, H), dtype=np.float32)
    for b in range(N_CORES):
        strip = res[b]["out"]  # [H+1, T]
        out[b] = (strip[:H, :] / strip[H : H + 1, :]).T
    return out


if __name__ == "__main__":
    rng = np.random.default_rng(0)
    x = rng.standard_normal((B, T, D)).astype(np.float32)
    s = 1.0 / np.sqrt(D)
    Wq = (rng.standard_normal((D, H)) * s).astype(np.float32)
    Wk = (rng.standard_normal((D, H)) * s).astype(np.float32)
    Wv = (rng.standard_normal((D, H)) * s).astype(np.float32)
    out = kernel(x=x, Wq=Wq, Wk=Wk, Wv=Wv)
    print("out", out.shape, out.dtype, np.abs(out).max())


# revision 21
# speedup vs baseline: 1.1124x; 1.1124x over previous
"""Single-head causal attention (B=8, T=2048, D=512, H=64) on 8 TRN2 cores.

Data-parallel: one batch element per NeuronCore. Each core computes
attention in the S^T layout (keys on partitions, queries on the free axis):

  qT/kT/vT [64, T] = W.T @ x.T        (f32r matmuls, N=512 chunks)
  v        [T, 64] via PE transpose of vT, with a ones column appended
  S^T[j,i] = kT_jblock.T @ qT          (strips of causal width)
  P^T      = exp(S^T / 8)              (ScalarE, one op per strip;
                                        no max-subtraction: scores are
                                        bounded by ~|q||k|sqrt(H)/8 << 88)
  out^T[h,i], l[i] = [v|1]_jb.T @ P^T  (accumulated over j-blocks in PSUM;
                                        row 64 is the softmax denominator)

The kernel returns the unnormalized [65, T] strip per core; the host
divides by the denominator row and transposes back to [T, 64].
"""

import sys

sys.path.insert(0, "/opt/trn_rl_repo")

import numpy as np

import concourse.bass as bass
import concourse.mybir as mybir
import concourse.tile as tile

B, T, D, H = 8, 2048, 512, 64
N_CORES = 8
HALF = T // 2  # i-axis pass width

f32 = mybir.dt.float32
f32r = mybir.dt.float32r
f16 = mybir.dt.float16

_cache = {}


def _legalize_waits(nc, max_waits=1):
    """Walrus codegen accepts at most one sync wait per instruction; hoist
    extras onto same-engine NOPs placed immediately before (engine queues
    are FIFO so blocking semantics are unchanged)."""
    counter = 0
    for bb in nc.main_func.blocks:
        if not any(
            ins.sync_info is not None and len(ins.sync_info.on_wait) > max_waits
            for ins in bb.instructions
        ):
            continue
        new_list = []
        for ins in bb.instructions:
            si = ins.sync_info
            if si is not None and len(si.on_wait) > max_waits:
                waits = list(si.on_wait)
                hoist, keep = waits[:-max_waits], waits[-max_waits:]
                for w in hoist:
                    counter += 1
                    new_list.append(
                        mybir.InstNoOp(
                            name=f"I-waitfix-{counter}",
                            engine=ins.engine,
                            sync_info=mybir.SyncInfo(on_wait=[w], on_update=[]),
                            bass_nofuse=True,
                        )
                    )
                ins.sync_info = mybir.SyncInfo(
                    on_wait=keep, on_update=list(si.on_update)
                )
            new_list.append(ins)
        bb.instructions = new_list
    return counter


def _chunks(lo, hi, step, align):
    """Split [lo, hi) at multiples of `step` relative to `align`."""
    out = []
    cur = lo
    while cur < hi:
        nxt = min(hi, align + ((cur - align) // step + 1) * step)
        out.append((cur, nxt))
        cur = nxt
    return out


def _build():
    nc = bass.Bass()

    xhi_d = nc.declare_dram_parameter("xhi", [D, T], f16, isOutput=False)
    # consts packed per partition (all fp16):
    # [wqk_hi c0..c3 | wv c0..c3 | mask | ones | ident]
    CW = 512 + 256 + 128 + 16 + 64  # 976
    consts_d = nc.declare_dram_parameter("consts", [128, CW], f16, isOutput=False)
    out_d = nc.declare_dram_parameter("out", [H + 1, T], f32, isOutput=True)

    NC_TILES = D // 128  # 4 c-tiles

    with tile.TileContext(nc) as tc:
        with (
            tc.tile_pool(name="const", bufs=1) as cpool,
            tc.tile_pool(name="xt", bufs=1) as xpool,
            tc.tile_pool(name="qkv", bufs=1) as qkvpool,
            tc.tile_pool(name="p", bufs=2) as ppool,
            tc.tile_pool(name="o", bufs=2) as opool,
            tc.tile_pool(name="ps_proj", bufs=2, space="PSUM") as ps_proj,
            tc.tile_pool(name="ps_s", bufs=2, space="PSUM") as ps_s,
            tc.tile_pool(name="ps_pv", bufs=1, space="PSUM") as ps_pv,
        ):
            consts = cpool.tile([128, CW], f16)
            nc.sync.dma_start(consts[:], consts_d[:])
            wqk_hi = [consts[:, 128 * c : 128 * (c + 1)] for c in range(NC_TILES)]
            wv = [
                consts[:, 512 + 64 * c : 512 + 64 * (c + 1)]
                for c in range(NC_TILES)
            ]
            mask16 = consts[:, 768:896]
            ones = consts[:, 896:912]
            ident16 = consts[0:H, 912:976]

            # initial HAM warm-up burst: one full SHORT window of dense bf16
            # matmuls while the input DMAs run, so the 2.4 GHz clock engages
            # before real work starts.
            warm_bf = cpool.tile([128, 512], mybir.dt.bfloat16)
            nc.vector.memset(warm_bf[:], 1.0)
            # touch Exp once so the ACT table set loads during the DMA phase
            exp_warm = cpool.tile([1, 2], f32)
            nc.scalar.activation(
                exp_warm[:], warm_bf[0:1, 0:2], mybir.ActivationFunctionType.Exp
            )
            warm_ps = ps_s.tile([128, 512], f32, tag="s", name="warm_ps")
            for _ in range(9):
                nc.tensor.matmul(
                    warm_ps[:], warm_bf[:, 0:128], warm_bf[:], start=True, stop=True
                )

            # host reorders x.T so DRAM row (4p + c) holds x.T row (128c + p):
            # one DMA per piece covers all four c-tiles with one 2D
            # descriptor per partition.
            xhi_all = xpool.tile([128, NC_TILES, T], f16)
            xhi_src = xhi_d.rearrange("(p c) t -> p c t", c=NC_TILES)
            xhi = [xhi_all[:, c, :] for c in range(NC_TILES)]
            qT = qkvpool.tile([H, T], f16)
            kT = qkvpool.tile([H, T], f16)
            vT = qkvpool.tile([H, T], f16)
            v1 = qkvpool.tile([128, T // 128, H + 1], f16)
            nc.vector.tensor_copy(v1[:, :, H : H + 1], ones)

            for lo, hi in ((0, 512), (512, 1024)):
                nc.sync.dma_start(xhi_all[:, :, lo:hi], xhi_src[:, :, lo:hi])
            nc.sync.dma_start(xhi_all[:, :, HALF:T], xhi_src[:, :, HALF:T])

            def proj_qk_subunits(tc512):
                # 3-pass split-fp16: Whi@xhi + Wlo@xhi + Whi@xlo, emitted as
                # three separately-schedulable sub-units sharing one psum
                # accumulation group
                state = {}

                def sub(pi, wgrp, xgrp):
                    if pi == 0:
                        state["ps"] = ps_proj.tile(
                            [128, 512], f32, tag="work", name="qk_ps"
                        )
                    qk_ps = state["ps"]
                    for c in range(NC_TILES):
                        nc.tensor.matmul(
                            qk_ps[:],
                            wgrp[c],
                            xgrp[c][:, tc512 : tc512 + 512],
                            start=(pi == 0 and c == 0),
                            stop=(pi == 0 and c == NC_TILES - 1),
                        )
                    if pi == 0:
                        nc.vector.tensor_copy(
                            qT[:, tc512 : tc512 + 512], qk_ps[0:H, :]
                        )
                        nc.vector.tensor_copy(
                            kT[:, tc512 : tc512 + 512], qk_ps[H : 2 * H, :]
                        )

                passes = [(wqk_hi, xhi)]
                return [
                    (lambda pi=pi, w=w, xg=xg: sub(pi, w, xg))
                    for pi, (w, xg) in enumerate(passes)
                ]

            def proj_qk_unit(tc512):
                for u in proj_qk_subunits(tc512):
                    u()

            def proj_v_unit(tc512):
                v_ps = ps_proj.tile([128, 512], f32, tag="work", name="v_ps")
                for c in range(NC_TILES):
                    nc.tensor.matmul(
                        v_ps[0:H, :],
                        wv[c],
                        xhi[c][:, tc512 : tc512 + 512],
                        start=(c == 0),
                        stop=(c == NC_TILES - 1),
                    )
                nc.vector.tensor_copy(vT[:, tc512 : tc512 + 512], v_ps[0:H, :])

            def vtrans_unit(jj_pair):
                vt_ps = ps_proj.tile([128, 2, H], f16, tag="work", name="vt_ps")
                for jl, jj in enumerate(jj_pair):
                    nc.tensor.transpose(
                        vt_ps[:, jl, :],
                        vT[:, 128 * jj : 128 * (jj + 1)],
                        ident16,
                    )
                    nc.vector.tensor_copy(v1[:, jj, 0:H], vt_ps[:, jl, :])

            def attn_S(t0, jb):
                # S^T strip matmuls for one j-block; emitted one iteration
                # ahead of its exp/PV so PV(jb-1)'s exp-wait never blocks
                # S(jb) in the PE FIFO
                i_start = max(t0, 128 * jb)
                W = t0 + HALF - i_start
                s_ps = ps_s.tile([128, HALF], f32, tag="s", name="s_ps")
                for ls, le in _chunks(0, W, 512, 0):
                    nc.tensor.matmul(
                        s_ps[:, ls:le],
                        kT[:, 128 * jb : 128 * (jb + 1)],
                        qT[:, i_start + ls : i_start + le],
                        start=True,
                        stop=True,
                    )
                return s_ps

            def attn_exp_pv(t0, n_jb, pv_ps, jb, s_ps):
                i_start = max(t0, 128 * jb)
                W = t0 + HALF - i_start
                p_sb = ppool.tile([128, HALF], f16, tag="p", name="p_sb", bufs=4)
                if t0 == 0 and jb <= 1 and W > 512:
                    # chain start: split the exp so it can begin as soon as
                    # the first qk chunk lands
                    nc.scalar.activation(
                        p_sb[:, 0:512],
                        s_ps[:, 0:512],
                        mybir.ActivationFunctionType.Exp,
                        scale=1.0 / 8.0,
                    )
                    nc.scalar.activation(
                        p_sb[:, 512:W],
                        s_ps[:, 512:W],
                        mybir.ActivationFunctionType.Exp,
                        scale=1.0 / 8.0,
                    )
                else:
                    nc.scalar.activation(
                        p_sb[:, 0:W],
                        s_ps[:, 0:W],
                        mybir.ActivationFunctionType.Exp,
                        scale=1.0 / 8.0,
                    )
                if 128 * jb >= t0:
                    nc.vector.tensor_mul(p_sb[:, 0:128], p_sb[:, 0:128], mask16)
                # PV accumulate: chunk by global-512 (pv bank) bounds
                for gs, ge in _chunks(i_start, t0 + HALF, 512, 0):
                    ic_last_jb = min(n_jb - 1, (ge - 1) // 128)
                    nc.tensor.matmul(
                        pv_ps[:, gs - t0 : ge - t0],
                        v1[:, jb, :],
                        p_sb[:, gs - i_start : ge - i_start],
                        start=(jb == 0),
                        stop=(jb == ic_last_jb),
                    )

            # --- phase 0: projections for half 0. S(0,0) is computed in
            # two halves interleaved with the projections so the exp
            # stream starts ~4us earlier (right after proj(tc0)) instead
            # of waiting for the full qT half ---
            proj_qk_unit(0)
            s00 = ps_s.tile([128, HALF], f32, tag="s", name="s_ps")
            nc.tensor.matmul(
                s00[:, 0:512], kT[:, 0:128], qT[:, 0:512], start=True, stop=True
            )
            p00 = ppool.tile([128, HALF], f16, tag="p", name="p_sb", bufs=4)
            nc.scalar.activation(
                p00[:, 0:512],
                s00[:, 0:512],
                mybir.ActivationFunctionType.Exp,
                scale=1.0 / 8.0,
            )
            nc.vector.tensor_mul(p00[:, 0:128], p00[:, 0:128], mask16)
            proj_v_unit(0)
            proj_qk_unit(512)
            nc.tensor.matmul(
                s00[:, 512:1024], kT[:, 0:128], qT[:, 512:1024], start=True, stop=True
            )
            nc.scalar.activation(
                p00[:, 512:1024],
                s00[:, 512:1024],
                mybir.ActivationFunctionType.Exp,
                scale=1.0 / 8.0,
            )
            proj_v_unit(512)
            for pair in ((0, 1), (2, 3), (4, 5), (6, 7)):
                vtrans_unit(pair)

            # --- attention pass 0, with half-1 projection units woven into
            # the PE stream to fill its exp-wait stalls ---
            h1_units = (
                [lambda: proj_v_unit(1024)]
                + proj_qk_subunits(1024)
                + [lambda: proj_v_unit(1536)]
                + proj_qk_subunits(1536)
                + [lambda: vtrans_unit((8, 9))]
            )
            def attn_pv_only(t0, n_jb, pv_ps, jb, p_sb):
                i_start = max(t0, 128 * jb)
                for gs, ge in _chunks(i_start, t0 + HALF, 512, 0):
                    ic_last_jb = min(n_jb - 1, (ge - 1) // 128)
                    nc.tensor.matmul(
                        pv_ps[:, gs - t0 : ge - t0],
                        v1[:, jb, :],
                        p_sb[:, gs - i_start : ge - i_start],
                        start=(jb == 0),
                        stop=(jb == ic_last_jb),
                    )

            def exp_store(t0, jb, s_ps):
                # exp into a held P slot (pass-1 strips precomputed during
                # pass-0's ACT-idle tail; no mask: these are non-diagonal)
                W = t0 + HALF - max(t0, 128 * jb)
                p_sb = ppool.tile([128, HALF], f16, tag="ppre", name="p_pre", bufs=6)
                nc.scalar.activation(
                    p_sb[:, 0:W],
                    s_ps[:, 0:W],
                    mybir.ActivationFunctionType.Exp,
                    scale=1.0 / 8.0,
                )
                return p_sb

            def out_piece(pv_ps, t0, lo, hi):
                out_sb = opool.tile(
                    [H + 1, 512], f32, tag="o", name="out_sb"
                )
                nc.vector.tensor_copy(out_sb[:, 0 : hi - lo], pv_ps[:, lo:hi])
                nc.sync.dma_start(
                    out_d[:, t0 + lo : t0 + hi], out_sb[:, 0 : hi - lo]
                )

            def out_chunk(pv_ps, t0, c):
                out_piece(pv_ps, t0, 512 * c, 512 * (c + 1))

            pv_ps0 = ps_pv.tile([H + 1, HALF], f32, tag="pv", name="pv_ps")
            s_cur = attn_S(0, 1)
            attn_pv_only(0, 8, pv_ps0, 0, p00)
            if h1_units:
                h1_units.pop(0)()
            for jb in range(1, 8):
                s_nxt = attn_S(0, jb + 1) if jb + 1 < 8 else None
                attn_exp_pv(0, 8, pv_ps0, jb, s_cur)
                s_cur = s_nxt
                if jb == 3:
                    out_chunk(pv_ps0, 0, 0)
                if h1_units:
                    h1_units.pop(0)()
            for u in h1_units:
                u()
            # precompute pass-1's first strips while ACT is otherwise idle
            pre_p = []
            for jbp in range(6):
                s_pre = attn_S(HALF, jbp)
                pre_p.append(exp_store(HALF, jbp, s_pre))
            out_chunk(pv_ps0, 0, 1)

            # --- attention pass 1 ---
            pv_ps1 = ps_pv.tile([H + 1, HALF], f32, tag="pv", name="pv_ps")
            s_cur = attn_S(HALF, 6)
            for jb in range(16):
                if jb < 6:
                    attn_pv_only(HALF, 16, pv_ps1, jb, pre_p[jb])
                    continue
                s_nxt = attn_S(HALF, jb + 1) if jb + 1 < 16 and jb + 1 > 6 else None
                attn_exp_pv(HALF, 16, pv_ps1, jb, s_cur)
                s_cur = s_nxt
                if jb == 11:
                    out_chunk(pv_ps1, HALF, 0)
                if jb == 8:
                    vtrans_unit((10, 11))
                elif jb == 9:
                    vtrans_unit((12, 13))
                elif jb == 10:
                    vtrans_unit((14, 15))
            out_chunk(pv_ps1, HALF, 1)

    _legalize_waits(nc)
    return nc


def build_in_maps(x, Wq, Wk, Wv):
    x = np.ascontiguousarray(np.asarray(x), dtype=np.float32)
    wqk_np = np.ascontiguousarray(
        np.concatenate([np.asarray(Wq), np.asarray(Wk)], axis=1), dtype=np.float32
    )
    wv_np = np.ascontiguousarray(np.asarray(Wv), dtype=np.float32)

    def ctile_pack(a, w):  # [512, w] -> [128, 4*w] with c-tiles side by side
        return a.reshape(4, 128, w).transpose(1, 0, 2).reshape(128, 4 * w)

    wqk_hi = wqk_np.astype(np.float16)
    mask_np = np.triu(np.ones((128, 128), dtype=np.float16))
    ident_np = np.zeros((128, H), dtype=np.float16)
    ident_np[:H] = np.eye(H, dtype=np.float16)
    ones_np = np.ones((128, T // 128), dtype=np.float16)
    consts_np = np.ascontiguousarray(
        np.concatenate(
            [
                ctile_pack(wqk_hi, 128),
                ctile_pack(wv_np.astype(np.float16), 64),
                mask_np,
                ones_np,
                ident_np,
            ],
            axis=1,
        )
    )

    def reorder(a):  # [512, T] -> row (4p + c) holds row (128c + p)
        return np.ascontiguousarray(
            a.reshape(4, 128, T).transpose(1, 0, 2).reshape(D, T)
        )

    maps = []
    for b in range(N_CORES):
        xhi = x[b].T.astype(np.float16)
        maps.append({"xhi": reorder(xhi), "consts": consts_np})
    return maps


def kernel(x, Wq, Wk, Wv):
    from concourse.bass_utils import run_bass_kernel_spmd

    if "nc" not in _cache:
        _cache["nc"] = _build()
    nc = _cache["nc"]

    in_maps = build_in_maps(x, Wq, Wk, Wv)
    res = run_bass_kernel_spmd(nc, in_maps, list(range(N_CORES))).results

    out = np.empty((B, T, H), dtype=np.float32)
    for b in range(N_CORES):
        strip = res[b]["out"]  # [H+1, T]
        out[b] = (strip[:H, :] / strip[H : H + 1, :]).T
    return out


if __name__ == "__main__":
    rng = np.random.default_rng(0)
    x = rng.standard_normal((B, T, D)).astype(np.float32)
    s = 1.0 / np.sqrt(D)
    Wq = (rng.standard_normal((D, H)) * s).astype(np.float32)
    Wk = (rng.standard_normal((D, H)) * s).astype(np.float32)
    Wv = (rng.standard_normal((D, H)) * s).astype(np.float32)
    out = kernel(x=x, Wq=Wq, Wk=Wk, Wv=Wv)
    print("out", out.shape, out.dtype, np.abs(out).max())


# revision 22
# speedup vs baseline: 1.2670x; 1.1390x over previous
"""Single-head causal attention (B=8, T=2048, D=512, H=64) on 8 TRN2 cores.

Data-parallel: one batch element per NeuronCore. Each core computes
attention in the S^T layout (keys on partitions, queries on the free axis):

  qT/kT/vT [64, T] = W.T @ x.T        (f32r matmuls, N=512 chunks)
  v        [T, 64] via PE transpose of vT, with a ones column appended
  S^T[j,i] = kT_jblock.T @ qT          (strips of causal width)
  P^T      = exp(S^T / 8)              (ScalarE, one op per strip;
                                        no max-subtraction: scores are
                                        bounded by ~|q||k|sqrt(H)/8 << 88)
  out^T[h,i], l[i] = [v|1]_jb.T @ P^T  (accumulated over j-blocks in PSUM;
                                        row 64 is the softmax denominator)

The kernel returns the unnormalized [65, T] strip per core; the host
divides by the denominator row and transposes back to [T, 64].
"""

import sys

sys.path.insert(0, "/opt/trn_rl_repo")

import numpy as np

import concourse.bass as bass
import concourse.mybir as mybir
import concourse.tile as tile

B, T, D, H = 8, 2048, 512, 64
N_CORES = 8
HALF = T // 2  # i-axis pass width

f32 = mybir.dt.float32
f32r = mybir.dt.float32r
f16 = mybir.dt.float16

_cache = {}


def _legalize_waits(nc, max_waits=1):
    """Walrus codegen accepts at most one sync wait per instruction; hoist
    extras onto same-engine NOPs placed immediately before (engine queues
    are FIFO so blocking semantics are unchanged)."""
    counter = 0
    for bb in nc.main_func.blocks:
        if not any(
            ins.sync_info is not None and len(ins.sync_info.on_wait) > max_waits
            for ins in bb.instructions
        ):
            continue
        new_list = []
        for ins in bb.instructions:
            si = ins.sync_info
            if si is not None and len(si.on_wait) > max_waits:
                waits = list(si.on_wait)
                hoist, keep = waits[:-max_waits], waits[-max_waits:]
                for w in hoist:
                    counter += 1
                    new_list.append(
                        mybir.InstNoOp(
                            name=f"I-waitfix-{counter}",
                            engine=ins.engine,
                            sync_info=mybir.SyncInfo(on_wait=[w], on_update=[]),
                            bass_nofuse=True,
                        )
                    )
                ins.sync_info = mybir.SyncInfo(
                    on_wait=keep, on_update=list(si.on_update)
                )
            new_list.append(ins)
        bb.instructions = new_list
    return counter


def _chunks(lo, hi, step, align):
    """Split [lo, hi) at multiples of `step` relative to `align`."""
    out = []
    cur = lo
    while cur < hi:
        nxt = min(hi, align + ((cur - align) // step + 1) * step)
        out.append((cur, nxt))
        cur = nxt
    return out


def _build():
    nc = bass.Bass()

    xhi_d = nc.declare_dram_parameter("xhi", [D, T], f16, isOutput=False)
    # consts packed per partition (all fp16):
    # [wqk_hi c0..c3 | wv c0..c3 | mask | ones | ident]
    CW = 512 + 256 + 128 + 16 + 64  # 976
    consts_d = nc.declare_dram_parameter("consts", [128, CW], f16, isOutput=False)
    out_d = nc.declare_dram_parameter("out", [H + 1, T], f32, isOutput=True)

    NC_TILES = D // 128  # 4 c-tiles

    with tile.TileContext(nc) as tc:
        with (
            tc.tile_pool(name="const", bufs=1) as cpool,
            tc.tile_pool(name="xt", bufs=1) as xpool,
            tc.tile_pool(name="qkv", bufs=1) as qkvpool,
            tc.tile_pool(name="p", bufs=2) as ppool,
            tc.tile_pool(name="o", bufs=2) as opool,
            tc.tile_pool(name="ps_proj", bufs=2, space="PSUM") as ps_proj,
            tc.tile_pool(name="ps_s", bufs=2, space="PSUM") as ps_s,
            tc.tile_pool(name="ps_pv", bufs=1, space="PSUM") as ps_pv,
        ):
            consts = cpool.tile([128, CW], f16)
            nc.sync.dma_start(consts[:], consts_d[:])
            wqk_hi = [consts[:, 128 * c : 128 * (c + 1)] for c in range(NC_TILES)]
            wv = [
                consts[:, 512 + 64 * c : 512 + 64 * (c + 1)]
                for c in range(NC_TILES)
            ]
            mask16 = consts[:, 768:896]
            ones = consts[:, 896:912]
            ident16 = consts[0:H, 912:976]

            # initial HAM warm-up burst: one full SHORT window of dense bf16
            # matmuls while the input DMAs run, so the 2.4 GHz clock engages
            # before real work starts.
            warm_bf = cpool.tile([128, 512], mybir.dt.bfloat16)
            nc.vector.memset(warm_bf[:], 1.0)
            # touch Exp once so the ACT table set loads during the DMA phase
            exp_warm = cpool.tile([1, 2], f32)
            nc.scalar.activation(
                exp_warm[:], warm_bf[0:1, 0:2], mybir.ActivationFunctionType.Exp
            )
            warm_ps = ps_s.tile([128, 512], f32, tag="s", name="warm_ps")
            for _ in range(9):
                nc.tensor.matmul(
                    warm_ps[:], warm_bf[:, 0:128], warm_bf[:], start=True, stop=True
                )

            # host reorders x.T so DRAM row (4p + c) holds x.T row (128c + p):
            # one DMA per piece covers all four c-tiles with one 2D
            # descriptor per partition.
            xhi_all = xpool.tile([128, NC_TILES, T], f16)
            xhi_src = xhi_d.rearrange("(p c) t -> p c t", c=NC_TILES)
            xhi = [xhi_all[:, c, :] for c in range(NC_TILES)]
            qT = qkvpool.tile([H, T], f16)
            kT = qkvpool.tile([H, T], f16)
            vT = qkvpool.tile([H, T], f16)
            v1 = qkvpool.tile([128, T // 128, H + 1], f16)
            nc.vector.tensor_copy(v1[:, :, H : H + 1], ones)

            for lo, hi in ((0, 512), (512, 1024)):
                nc.sync.dma_start(xhi_all[:, :, lo:hi], xhi_src[:, :, lo:hi])
            nc.sync.dma_start(xhi_all[:, :, HALF:T], xhi_src[:, :, HALF:T])

            def proj_qk_subunits(tc512):
                # 3-pass split-fp16: Whi@xhi + Wlo@xhi + Whi@xlo, emitted as
                # three separately-schedulable sub-units sharing one psum
                # accumulation group
                state = {}

                def sub(pi, wgrp, xgrp):
                    if pi == 0:
                        state["ps"] = ps_proj.tile(
                            [128, 512], f32, tag="work", name="qk_ps"
                        )
                    qk_ps = state["ps"]
                    for c in range(NC_TILES):
                        nc.tensor.matmul(
                            qk_ps[:],
                            wgrp[c],
                            xgrp[c][:, tc512 : tc512 + 512],
                            start=(pi == 0 and c == 0),
                            stop=(pi == 0 and c == NC_TILES - 1),
                        )
                    if pi == 0:
                        nc.vector.tensor_copy(
                            qT[:, tc512 : tc512 + 512], qk_ps[0:H, :]
                        )
                        nc.vector.tensor_copy(
                            kT[:, tc512 : tc512 + 512], qk_ps[H : 2 * H, :]
                        )

                passes = [(wqk_hi, xhi)]
                return [
                    (lambda pi=pi, w=w, xg=xg: sub(pi, w, xg))
                    for pi, (w, xg) in enumerate(passes)
                ]

            def proj_qk_unit(tc512):
                for u in proj_qk_subunits(tc512):
                    u()

            def proj_v_unit(tc512):
                v_ps = ps_proj.tile([128, 512], f32, tag="work", name="v_ps")
                for c in range(NC_TILES):
                    nc.tensor.matmul(
                        v_ps[0:H, :],
                        wv[c],
                        xhi[c][:, tc512 : tc512 + 512],
                        start=(c == 0),
                        stop=(c == NC_TILES - 1),
                    )
                nc.vector.tensor_copy(vT[:, tc512 : tc512 + 512], v_ps[0:H, :])

            def vtrans_unit(jj_pair):
                vt_ps = ps_proj.tile([128, 2, H], f16, tag="work", name="vt_ps")
                for jl, jj in enumerate(jj_pair):
                    nc.tensor.transpose(
                        vt_ps[:, jl, :],
                        vT[:, 128 * jj : 128 * (jj + 1)],
                        ident16,
                    )
                    nc.vector.tensor_copy(v1[:, jj, 0:H], vt_ps[:, jl, :])

            def attn_S(t0, jb):
                # S^T strip matmuls for one j-block; emitted one iteration
                # ahead of its exp/PV so PV(jb-1)'s exp-wait never blocks
                # S(jb) in the PE FIFO
                i_start = max(t0, 128 * jb)
                W = t0 + HALF - i_start
                s_ps = ps_s.tile([128, HALF], f32, tag="s", name="s_ps")
                for ls, le in _chunks(0, W, 512, 0):
                    nc.tensor.matmul(
                        s_ps[:, ls:le],
                        kT[:, 128 * jb : 128 * (jb + 1)],
                        qT[:, i_start + ls : i_start + le],
                        start=True,
                        stop=True,
                    )
                return s_ps

            def attn_exp_pv(t0, n_jb, pv_ps, jb, s_ps):
                i_start = max(t0, 128 * jb)
                W = t0 + HALF - i_start
                p_sb = ppool.tile([128, HALF], f16, tag="p", name="p_sb", bufs=4)
                if t0 == 0 and jb <= 1 and W > 512:
                    # chain start: split the exp so it can begin as soon as
                    # the first qk chunk lands
                    nc.scalar.activation(
                        p_sb[:, 0:512],
                        s_ps[:, 0:512],
                        mybir.ActivationFunctionType.Exp,
                        scale=1.0 / 8.0,
                    )
                    nc.scalar.activation(
                        p_sb[:, 512:W],
                        s_ps[:, 512:W],
                        mybir.ActivationFunctionType.Exp,
                        scale=1.0 / 8.0,
                    )
                else:
                    nc.scalar.activation(
                        p_sb[:, 0:W],
                        s_ps[:, 0:W],
                        mybir.ActivationFunctionType.Exp,
                        scale=1.0 / 8.0,
                    )
                if 128 * jb >= t0:
                    nc.vector.tensor_mul(p_sb[:, 0:128], p_sb[:, 0:128], mask16)
                # PV accumulate: chunk by global-512 (pv bank) bounds
                for gs, ge in _chunks(i_start, t0 + HALF, 512, 0):
                    ic_last_jb = min(n_jb - 1, (ge - 1) // 128)
                    nc.tensor.matmul(
                        pv_ps[:, gs - t0 : ge - t0],
                        v1[:, jb, :],
                        p_sb[:, gs - i_start : ge - i_start],
                        start=(jb == 0),
                        stop=(jb == ic_last_jb),
                    )

            # --- phase 0: projections for half 0 ---
            for tc512 in (0, 512):
                proj_qk_unit(tc512)
                proj_v_unit(tc512)
            for pair in ((0, 1), (2, 3), (4, 5), (6, 7)):
                vtrans_unit(pair)

            # --- attention pass 0, with half-1 projection units woven into
            # the PE stream to fill its exp-wait stalls ---
            h1_units = (
                [lambda: proj_v_unit(1024)]
                + proj_qk_subunits(1024)
                + [lambda: proj_v_unit(1536)]
                + proj_qk_subunits(1536)
                + [lambda: vtrans_unit((8, 9))]
            )
            def attn_pv_only(t0, n_jb, pv_ps, jb, p_sb):
                i_start = max(t0, 128 * jb)
                for gs, ge in _chunks(i_start, t0 + HALF, 512, 0):
                    ic_last_jb = min(n_jb - 1, (ge - 1) // 128)
                    nc.tensor.matmul(
                        pv_ps[:, gs - t0 : ge - t0],
                        v1[:, jb, :],
                        p_sb[:, gs - i_start : ge - i_start],
                        start=(jb == 0),
                        stop=(jb == ic_last_jb),
                    )

            def exp_store(t0, jb, s_ps):
                # exp into a held P slot (pass-1 strips precomputed during
                # pass-0's ACT-idle tail; no mask: these are non-diagonal)
                W = t0 + HALF - max(t0, 128 * jb)
                p_sb = ppool.tile([128, HALF], f16, tag="ppre", name="p_pre", bufs=6)
                nc.scalar.activation(
                    p_sb[:, 0:W],
                    s_ps[:, 0:W],
                    mybir.ActivationFunctionType.Exp,
                    scale=1.0 / 8.0,
                )
                return p_sb

            def out_piece(pv_ps, t0, lo, hi):
                out_sb = opool.tile(
                    [H + 1, 512], f32, tag="o", name="out_sb"
                )
                nc.vector.tensor_copy(out_sb[:, 0 : hi - lo], pv_ps[:, lo:hi])
                nc.sync.dma_start(
                    out_d[:, t0 + lo : t0 + hi], out_sb[:, 0 : hi - lo]
                )

            def out_chunk(pv_ps, t0, c):
                out_piece(pv_ps, t0, 512 * c, 512 * (c + 1))

            pv_ps0 = ps_pv.tile([H + 1, HALF], f32, tag="pv", name="pv_ps")
            s_cur = attn_S(0, 0)
            for jb in range(8):
                s_nxt = attn_S(0, jb + 1) if jb + 1 < 8 else None
                attn_exp_pv(0, 8, pv_ps0, jb, s_cur)
                s_cur = s_nxt
                if jb == 3:
                    out_chunk(pv_ps0, 0, 0)
                if h1_units:
                    h1_units.pop(0)()
            for u in h1_units:
                u()
            # precompute pass-1's first strips while ACT is otherwise idle
            pre_p = []
            for jbp in range(6):
                s_pre = attn_S(HALF, jbp)
                pre_p.append(exp_store(HALF, jbp, s_pre))
            out_chunk(pv_ps0, 0, 1)

            # --- attention pass 1 ---
            pv_ps1 = ps_pv.tile([H + 1, HALF], f32, tag="pv", name="pv_ps")
            s_cur = attn_S(HALF, 6)
            for jb in range(16):
                if jb < 6:
                    attn_pv_only(HALF, 16, pv_ps1, jb, pre_p[jb])
                    continue
                s_nxt = attn_S(HALF, jb + 1) if jb + 1 < 16 and jb + 1 > 6 else None
                attn_exp_pv(HALF, 16, pv_ps1, jb, s_cur)
                s_cur = s_nxt
                if jb == 11:
                    out_chunk(pv_ps1, HALF, 0)
                if jb == 8:
                    vtrans_unit((10, 11))
                elif jb == 9:
                    vtrans_unit((12, 13))
                elif jb == 10:
                    vtrans_unit((14, 15))
            out_chunk(pv_ps1, HALF, 1)

    _legalize_waits(nc)
    return nc


def build_in_maps(x, Wq, Wk, Wv):
    x = np.ascontiguousarray(np.asarray(x), dtype=np.float32)
    wqk_np = np.ascontiguousarray(
        np.concatenate([np.asarray(Wq), np.asarray(Wk)], axis=1), dtype=np.float32
    )
    wv_np = np.ascontiguousarray(np.asarray(Wv), dtype=np.float32)

    def ctile_pack(a, w):  # [512, w] -> [128, 4*w] with c-tiles side by side
        return a.reshape(4, 128, w).transpose(1, 0, 2).reshape(128, 4 * w)

    wqk_hi = wqk_np.astype(np.float16)
    mask_np = np.triu(np.ones((128, 128), dtype=np.float16))
    ident_np = np.zeros((128, H), dtype=np.float16)
    ident_np[:H] = np.eye(H, dtype=np.float16)
    ones_np = np.ones((128, T // 128), dtype=np.float16)
    consts_np = np.ascontiguousarray(
        np.concatenate(
            [
                ctile_pack(wqk_hi, 128),
                ctile_pack(wv_np.astype(np.float16), 64),
                mask_np,
                ones_np,
                ident_np,
            ],
            axis=1,
        )
    )

    def reorder(a):  # [512, T] -> row (4p + c) holds row (128c + p)
        return np.ascontiguousarray(
            a.reshape(4, 128, T).transpose(1, 0, 2).reshape(D, T)
        )

    maps = []
    for b in range(N_CORES):
        xhi = x[b].T.astype(np.float16)
        maps.append({"xhi": reorder(xhi), "consts": consts_np})
    return maps


def kernel(x, Wq, Wk, Wv):
    from concourse.bass_utils import run_bass_kernel_spmd

    if "nc" not in _cache:
        _cache["nc"] = _build()
    nc = _cache["nc"]

    in_maps = build_in_maps(x, Wq, Wk, Wv)
    res = run_bass_kernel_spmd(nc, in_maps, list(range(N_CORES))).results

    out = np.empty((B, T, H), dtype=np.float32)
    for b in range(N_CORES):
        strip = res[b]["out"]  # [H+1, T]
        out[b] = (strip[:H, :] / strip[H : H + 1, :]).T
    return out


if __name__ == "__main__":
    rng = np.random.default_rng(0)
    x = rng.standard_normal((B, T, D)).astype(np.float32)
    s = 1.0 / np.sqrt(D)
    Wq = (rng.standard_normal((D, H)) * s).astype(np.float32)
    Wk = (rng.standard_normal((D, H)) * s).astype(np.float32)
    Wv = (rng.standard_normal((D, H)) * s).astype(np.float32)
    out = kernel(x=x, Wq=Wq, Wk=Wk, Wv=Wv)
    print("out", out.shape, out.dtype, np.abs(out).max())


# revision 25
# speedup vs baseline: 1.2977x; 1.0242x over previous
"""Single-head causal attention (B=8, T=2048, D=512, H=64) on 8 TRN2 cores.

Data-parallel: one batch element per NeuronCore. Each core computes
attention in the S^T layout (keys on partitions, queries on the free axis):

  qT/kT/vT [64, T] = W.T @ x.T        (f32r matmuls, N=512 chunks)
  v        [T, 64] via PE transpose of vT, with a ones column appended
  S^T[j,i] = kT_jblock.T @ qT          (strips of causal width)
  P^T      = exp(S^T / 8)              (ScalarE, one op per strip;
                                        no max-subtraction: scores are
                                        bounded by ~|q||k|sqrt(H)/8 << 88)
  out^T[h,i], l[i] = [v|1]_jb.T @ P^T  (accumulated over j-blocks in PSUM;
                                        row 64 is the softmax denominator)

The kernel returns the unnormalized [65, T] strip per core; the host
divides by the denominator row and transposes back to [T, 64].
"""

import sys

sys.path.insert(0, "/opt/trn_rl_repo")

import numpy as np

import concourse.bass as bass
import concourse.mybir as mybir
import concourse.tile as tile

B, T, D, H = 8, 2048, 512, 64
N_CORES = 8
HALF = T // 2  # i-axis pass width

f32 = mybir.dt.float32
f32r = mybir.dt.float32r
f16 = mybir.dt.float16

_cache = {}


def _legalize_waits(nc, max_waits=1):
    """Walrus codegen accepts at most one sync wait per instruction; hoist
    extras onto same-engine NOPs placed immediately before (engine queues
    are FIFO so blocking semantics are unchanged)."""
    counter = 0
    for bb in nc.main_func.blocks:
        if not any(
            ins.sync_info is not None and len(ins.sync_info.on_wait) > max_waits
            for ins in bb.instructions
        ):
            continue
        new_list = []
        for ins in bb.instructions:
            si = ins.sync_info
            if si is not None and len(si.on_wait) > max_waits:
                waits = list(si.on_wait)
                hoist, keep = waits[:-max_waits], waits[-max_waits:]
                for w in hoist:
                    counter += 1
                    new_list.append(
                        mybir.InstNoOp(
                            name=f"I-waitfix-{counter}",
                            engine=ins.engine,
                            sync_info=mybir.SyncInfo(on_wait=[w], on_update=[]),
                            bass_nofuse=True,
                        )
                    )
                ins.sync_info = mybir.SyncInfo(
                    on_wait=keep, on_update=list(si.on_update)
                )
            new_list.append(ins)
        bb.instructions = new_list
    return counter


def _chunks(lo, hi, step, align):
    """Split [lo, hi) at multiples of `step` relative to `align`."""
    out = []
    cur = lo
    while cur < hi:
        nxt = min(hi, align + ((cur - align) // step + 1) * step)
        out.append((cur, nxt))
        cur = nxt
    return out


def _build():
    nc = bass.Bass()

    xhi_d = nc.declare_dram_parameter("xhi", [D, T], f16, isOutput=False)
    # consts packed per partition (all fp16):
    # [wqk_hi c0..c3 | wv c0..c3 | mask | ones | ident]
    CW = 512 + 256 + 128 + 16 + 64  # 976
    consts_d = nc.declare_dram_parameter("consts", [128, CW], f16, isOutput=False)
    out_d = nc.declare_dram_parameter("out", [H + 1, T], f32, isOutput=True)

    NC_TILES = D // 128  # 4 c-tiles

    with tile.TileContext(nc) as tc:
        with (
            tc.tile_pool(name="const", bufs=1) as cpool,
            tc.tile_pool(name="xt", bufs=1) as xpool,
            tc.tile_pool(name="qkv", bufs=1) as qkvpool,
            tc.tile_pool(name="p", bufs=2) as ppool,
            tc.tile_pool(name="o", bufs=2) as opool,
            tc.tile_pool(name="ps_proj", bufs=2, space="PSUM") as ps_proj,
            tc.tile_pool(name="ps_s", bufs=2, space="PSUM") as ps_s,
            tc.tile_pool(name="ps_pv", bufs=1, space="PSUM") as ps_pv,
        ):
            consts = cpool.tile([128, CW], f16)
            nc.sync.dma_start(consts[:], consts_d[:])
            wqk_hi = [consts[:, 128 * c : 128 * (c + 1)] for c in range(NC_TILES)]
            wv = [
                consts[:, 512 + 64 * c : 512 + 64 * (c + 1)]
                for c in range(NC_TILES)
            ]
            mask16 = consts[:, 768:896]
            ones = consts[:, 896:912]
            ident16 = consts[0:H, 912:976]

            # initial HAM warm-up burst: one full SHORT window of dense bf16
            # matmuls while the input DMAs run, so the 2.4 GHz clock engages
            # before real work starts.
            warm_bf = cpool.tile([128, 512], mybir.dt.bfloat16)
            nc.vector.memset(warm_bf[:], 1.0)
            # touch Exp once so the ACT table set loads during the DMA phase
            exp_warm = cpool.tile([1, 2], f32)
            nc.scalar.activation(
                exp_warm[:], warm_bf[0:1, 0:2], mybir.ActivationFunctionType.Exp
            )
            warm_ps = ps_s.tile([128, 512], f32, tag="s", name="warm_ps")
            for _ in range(9):
                nc.tensor.matmul(
                    warm_ps[:], warm_bf[:, 0:128], warm_bf[:], start=True, stop=True
                )

            # host reorders x.T so DRAM row (4p + c) holds x.T row (128c + p):
            # one DMA per piece covers all four c-tiles with one 2D
            # descriptor per partition.
            xhi_all = xpool.tile([128, NC_TILES, T], f16)
            xhi_src = xhi_d.rearrange("(p c) t -> p c t", c=NC_TILES)
            xhi = [xhi_all[:, c, :] for c in range(NC_TILES)]
            qT = qkvpool.tile([H, T], f16)
            kT = qkvpool.tile([H, T], f16)
            vT = qkvpool.tile([H, T], f16)
            v1 = qkvpool.tile([128, T // 128, H + 1], f16)
            nc.vector.tensor_copy(v1[:, :, H : H + 1], ones)

            for lo, hi in ((0, 512), (512, 1024)):
                nc.sync.dma_start(xhi_all[:, :, lo:hi], xhi_src[:, :, lo:hi])
            nc.sync.dma_start(xhi_all[:, :, HALF:T], xhi_src[:, :, HALF:T])

            def proj_qk_subunits(tc512):
                # 3-pass split-fp16: Whi@xhi + Wlo@xhi + Whi@xlo, emitted as
                # three separately-schedulable sub-units sharing one psum
                # accumulation group
                state = {}

                def sub(pi, wgrp, xgrp):
                    if pi == 0:
                        state["ps"] = ps_proj.tile(
                            [128, 512], f32, tag="work", name="qk_ps"
                        )
                    qk_ps = state["ps"]
                    for c in range(NC_TILES):
                        nc.tensor.matmul(
                            qk_ps[:],
                            wgrp[c],
                            xgrp[c][:, tc512 : tc512 + 512],
                            start=(pi == 0 and c == 0),
                            stop=(pi == 0 and c == NC_TILES - 1),
                        )
                    if pi == 0:
                        nc.vector.tensor_copy(
                            qT[:, tc512 : tc512 + 512], qk_ps[0:H, :]
                        )
                        nc.vector.tensor_copy(
                            kT[:, tc512 : tc512 + 512], qk_ps[H : 2 * H, :]
                        )

                passes = [(wqk_hi, xhi)]
                return [
                    (lambda pi=pi, w=w, xg=xg: sub(pi, w, xg))
                    for pi, (w, xg) in enumerate(passes)
                ]

            def proj_qk_unit(tc512):
                for u in proj_qk_subunits(tc512):
                    u()

            def proj_v_unit(tc512):
                v_ps = ps_proj.tile([128, 512], f32, tag="work", name="v_ps")
                for c in range(NC_TILES):
                    nc.tensor.matmul(
                        v_ps[0:H, :],
                        wv[c],
                        xhi[c][:, tc512 : tc512 + 512],
                        start=(c == 0),
                        stop=(c == NC_TILES - 1),
                    )
                nc.vector.tensor_copy(vT[:, tc512 : tc512 + 512], v_ps[0:H, :])

            def vtrans_unit(jj_pair):
                vt_ps = ps_proj.tile([128, 2, H], f16, tag="work", name="vt_ps")
                for jl, jj in enumerate(jj_pair):
                    nc.tensor.transpose(
                        vt_ps[:, jl, :],
                        vT[:, 128 * jj : 128 * (jj + 1)],
                        ident16,
                    )
                    nc.vector.tensor_copy(v1[:, jj, 0:H], vt_ps[:, jl, :])

            def attn_S(t0, jb):
                # S^T strip matmuls for one j-block; emitted one iteration
                # ahead of its exp/PV so PV(jb-1)'s exp-wait never blocks
                # S(jb) in the PE FIFO
                i_start = max(t0, 128 * jb)
                W = t0 + HALF - i_start
                s_ps = ps_s.tile([128, HALF], f32, tag="s", name="s_ps")
                for ls, le in _chunks(0, W, 512, 0):
                    nc.tensor.matmul(
                        s_ps[:, ls:le],
                        kT[:, 128 * jb : 128 * (jb + 1)],
                        qT[:, i_start + ls : i_start + le],
                        start=True,
                        stop=True,
                    )
                return s_ps

            def attn_S2(t0, jbA, jbB):
                # two short tail strips packed side by side in one psum tile
                # so a single exp instruction (250ns fixed cost) covers both
                s_ps = ps_s.tile([128, HALF], f32, tag="s", name="s_ps")
                WA = t0 + HALF - max(t0, 128 * jbA)
                WB = t0 + HALF - max(t0, 128 * jbB)
                for jb, off, W in ((jbA, 0, WA), (jbB, WA, WB)):
                    i_start = max(t0, 128 * jb)
                    for ls, le in _chunks(off, off + W, 512, 0):
                        nc.tensor.matmul(
                            s_ps[:, ls:le],
                            kT[:, 128 * jb : 128 * (jb + 1)],
                            qT[:, i_start - off + ls : i_start - off + le],
                            start=True,
                            stop=True,
                        )
                return s_ps

            def attn_exp_pv2(t0, n_jb, pv_ps, jbA, jbB, s_ps):
                WA = t0 + HALF - max(t0, 128 * jbA)
                WB = t0 + HALF - max(t0, 128 * jbB)
                p_sb = ppool.tile([128, HALF], f16, tag="p", name="p_sb", bufs=4)
                nc.scalar.activation(
                    p_sb[:, 0 : WA + WB],
                    s_ps[:, 0 : WA + WB],
                    mybir.ActivationFunctionType.Exp,
                    scale=1.0 / 8.0,
                )
                for jb, off in ((jbA, 0), (jbB, WA)):
                    if 128 * jb >= t0:
                        nc.vector.tensor_mul(
                            p_sb[:, off : off + 128],
                            p_sb[:, off : off + 128],
                            mask16,
                        )
                for jb, off in ((jbA, 0), (jbB, WA)):
                    i_start = max(t0, 128 * jb)
                    for gs, ge in _chunks(i_start, t0 + HALF, 512, 0):
                        ic_last_jb = min(n_jb - 1, (ge - 1) // 128)
                        nc.tensor.matmul(
                            pv_ps[:, gs - t0 : ge - t0],
                            v1[:, jb, :],
                            p_sb[:, off + gs - i_start : off + ge - i_start],
                            start=(jb == 0),
                            stop=(jb == ic_last_jb),
                        )

            def attn_exp_pv(t0, n_jb, pv_ps, jb, s_ps):
                i_start = max(t0, 128 * jb)
                W = t0 + HALF - i_start
                p_sb = ppool.tile([128, HALF], f16, tag="p", name="p_sb", bufs=4)
                if t0 == 0 and jb <= 1 and W > 512:
                    # chain start: split the exp so it can begin as soon as
                    # the first qk chunk lands
                    nc.scalar.activation(
                        p_sb[:, 0:512],
                        s_ps[:, 0:512],
                        mybir.ActivationFunctionType.Exp,
                        scale=1.0 / 8.0,
                    )
                    nc.scalar.activation(
                        p_sb[:, 512:W],
                        s_ps[:, 512:W],
                        mybir.ActivationFunctionType.Exp,
                        scale=1.0 / 8.0,
                    )
                else:
                    nc.scalar.activation(
                        p_sb[:, 0:W],
                        s_ps[:, 0:W],
                        mybir.ActivationFunctionType.Exp,
                        scale=1.0 / 8.0,
                    )
                if 128 * jb >= t0:
                    nc.vector.tensor_mul(p_sb[:, 0:128], p_sb[:, 0:128], mask16)
                # PV accumulate: chunk by global-512 (pv bank) bounds
                for gs, ge in _chunks(i_start, t0 + HALF, 512, 0):
                    ic_last_jb = min(n_jb - 1, (ge - 1) // 128)
                    nc.tensor.matmul(
                        pv_ps[:, gs - t0 : ge - t0],
                        v1[:, jb, :],
                        p_sb[:, gs - i_start : ge - i_start],
                        start=(jb == 0),
                        stop=(jb == ic_last_jb),
                    )

            # --- phase 0: projections for half 0 ---
            for tc512 in (0, 512):
                proj_qk_unit(tc512)
                proj_v_unit(tc512)
            for pair in ((0, 1), (2, 3), (4, 5), (6, 7)):
                vtrans_unit(pair)

            # --- attention pass 0, with half-1 projection units woven into
            # the PE stream to fill its exp-wait stalls ---
            h1_units = (
                [lambda: proj_v_unit(1024)]
                + proj_qk_subunits(1024)
                + [lambda: proj_v_unit(1536)]
                + proj_qk_subunits(1536)
                + [lambda: vtrans_unit((8, 9))]
            )
            def attn_pv_only(t0, n_jb, pv_ps, jb, p_sb):
                i_start = max(t0, 128 * jb)
                for gs, ge in _chunks(i_start, t0 + HALF, 512, 0):
                    ic_last_jb = min(n_jb - 1, (ge - 1) // 128)
                    nc.tensor.matmul(
                        pv_ps[:, gs - t0 : ge - t0],
                        v1[:, jb, :],
                        p_sb[:, gs - i_start : ge - i_start],
                        start=(jb == 0),
                        stop=(jb == ic_last_jb),
                    )

            def exp_store(t0, jb, s_ps):
                # exp into a held P slot (pass-1 strips precomputed during
                # pass-0's ACT-idle tail; no mask: these are non-diagonal)
                W = t0 + HALF - max(t0, 128 * jb)
                p_sb = ppool.tile([128, HALF], f16, tag="ppre", name="p_pre", bufs=6)
                nc.scalar.activation(
                    p_sb[:, 0:W],
                    s_ps[:, 0:W],
                    mybir.ActivationFunctionType.Exp,
                    scale=1.0 / 8.0,
                )
                return p_sb

            def out_piece(pv_ps, t0, lo, hi):
                out_sb = opool.tile(
                    [H + 1, 512], f32, tag="o", name="out_sb"
                )
                nc.vector.tensor_copy(out_sb[:, 0 : hi - lo], pv_ps[:, lo:hi])
                nc.sync.dma_start(
                    out_d[:, t0 + lo : t0 + hi], out_sb[:, 0 : hi - lo]
                )

            def out_chunk(pv_ps, t0, c):
                out_piece(pv_ps, t0, 512 * c, 512 * (c + 1))

            pv_ps0 = ps_pv.tile([H + 1, HALF], f32, tag="pv", name="pv_ps")
            s_cur = attn_S(0, 0)
            for jb in range(6):
                s_nxt = attn_S(0, jb + 1) if jb + 1 < 6 else attn_S2(0, 6, 7)
                attn_exp_pv(0, 8, pv_ps0, jb, s_cur)
                s_cur = s_nxt
                if jb == 3:
                    out_chunk(pv_ps0, 0, 0)
                if h1_units:
                    h1_units.pop(0)()
            attn_exp_pv2(0, 8, pv_ps0, 6, 7, s_cur)
            for u in h1_units:
                u()
            # precompute pass-1's first strips while ACT is otherwise idle
            pre_p = []
            for jbp in range(6):
                s_pre = attn_S(HALF, jbp)
                pre_p.append(exp_store(HALF, jbp, s_pre))
            out_chunk(pv_ps0, 0, 1)

            # --- attention pass 1 ---
            pv_ps1 = ps_pv.tile([H + 1, HALF], f32, tag="pv", name="pv_ps")
            s_cur = attn_S(HALF, 6)
            for jb in range(12):
                if jb < 6:
                    attn_pv_only(HALF, 16, pv_ps1, jb, pre_p[jb])
                    continue
                s_nxt = (
                    attn_S(HALF, jb + 1)
                    if jb + 1 <= 11
                    else attn_S2(HALF, 12, 13)
                )
                attn_exp_pv(HALF, 16, pv_ps1, jb, s_cur)
                s_cur = s_nxt
                if jb == 11:
                    out_chunk(pv_ps1, HALF, 0)
                if jb == 8:
                    vtrans_unit((10, 11))
                elif jb == 9:
                    vtrans_unit((12, 13))
                elif jb == 10:
                    vtrans_unit((14, 15))
            s1415 = attn_S2(HALF, 14, 15)
            attn_exp_pv2(HALF, 16, pv_ps1, 12, 13, s_cur)
            attn_exp_pv2(HALF, 16, pv_ps1, 14, 15, s1415)
            out_chunk(pv_ps1, HALF, 1)

    _legalize_waits(nc)
    return nc


def build_in_maps(x, Wq, Wk, Wv):
    x = np.ascontiguousarray(np.asarray(x), dtype=np.float32)
    wqk_np = np.ascontiguousarray(
        np.concatenate([np.asarray(Wq), np.asarray(Wk)], axis=1), dtype=np.float32
    )
    wv_np = np.ascontiguousarray(np.asarray(Wv), dtype=np.float32)

    def ctile_pack(a, w):  # [512, w] -> [128, 4*w] with c-tiles side by side
        return a.reshape(4, 128, w).transpose(1, 0, 2).reshape(128, 4 * w)

    wqk_hi = wqk_np.astype(np.float16)
    mask_np = np.triu(np.ones((128, 128), dtype=np.float16))
    ident_np = np.zeros((128, H), dtype=np.float16)
    ident_np[:H] = np.eye(H, dtype=np.float16)
    ones_np = np.ones((128, T // 128), dtype=np.float16)
    consts_np = np.ascontiguousarray(
        np.concatenate(
            [
                ctile_pack(wqk_hi, 128),
                ctile_pack(wv_np.astype(np.float16), 64),
                mask_np,
                ones_np,
                ident_np,
            ],
            axis=1,
        )
    )

    def reorder(a):  # [512, T] -> row (4p + c) holds row (128c + p)
        return np.ascontiguousarray(
            a.reshape(4, 128, T).transpose(1, 0, 2).reshape(D, T)
        )

    maps = []
    for b in range(N_CORES):
        xhi = x[b].T.astype(np.float16)
        maps.append({"xhi": reorder(xhi), "consts": consts_np})
    return maps


def kernel(x, Wq, Wk, Wv):
    from concourse.bass_utils import run_bass_kernel_spmd

    if "nc" not in _cache:
        _cache["nc"] = _build()
    nc = _cache["nc"]

    in_maps = build_in_maps(x, Wq, Wk, Wv)
    res = run_bass_kernel_spmd(nc, in_maps, list(range(N_CORES))).results

    out = np.empty((B, T, H), dtype=np.float32)
    for b in range(N_CORES):
        strip = res[b]["out"]  # [H+1, T]
        out[b] = (strip[:H, :] / strip[H : H + 1, :]).T
    return out


if __name__ == "__main__":
    rng = np.random.default_rng(0)
    x = rng.standard_normal((B, T, D)).astype(np.float32)
    s = 1.0 / np.sqrt(D)
    Wq = (rng.standard_normal((D, H)) * s).astype(np.float32)
    Wk = (rng.standard_normal((D, H)) * s).astype(np.float32)
    Wv = (rng.standard_normal((D, H)) * s).astype(np.float32)
    out = kernel(x=x, Wq=Wq, Wk=Wk, Wv=Wv)
    print("out", out.shape, out.dtype, np.abs(out).max())
